# revision 21
# baseline (speedup 1.0000x reference)
"""ChebyNet (K=3, 2 layers) forward on 8 Trainium2 NeuronCores.

Strategy: node sharding. Each core owns 1280 padded rows (10000 -> 10240).
The sparse propagation  L = -D^-1/2 A D^-1/2  is computed as a dense matmul
against the transposed adjacency-count matrix AT[s, d], held SBUF-resident in
fp8e4m3 (counts are small ints -> exact, partition-contiguous DRAM layout for
fast load). Features move in bf16, accumulation in fp32 PSUM, diagonal
scalings as per-partition scalar multiplies on the vector engine. Between
hops the scaled features are AllGathered across the 8 cores; each AllGather
is split into two half-shard collectives overlapped with compute.

Both layers are restructured using linearity of L (it commutes with the
feature-dimension matmuls), so each hop propagates the minimum column count
and layer 1 needs no on-device transposes:

  Layer 1:  h = relu( x(W10-W12) + L( x W11 + L(x 2W12) ) + b1 )
     d1 = x@W11, d2 = x@(2 W12), e0 = x@(W10-W12)   (from host-side x^T)
     hop A: Ld2 = L d2      (256 cols)   s1 = d1 + Ld2
     hop B: Ls1 = L s1      (256 cols)   h = relu(e0 + Ls1 + b1)
  Layer 2:  out = h(W20-W22) + L( h W21 + L(h 2W22) ) + b2
     z1 = h@W21, z2 = h@(2 W22), hw = h@(W20-W22)   (from PE-transposed h)
     hop C: Lz2 = L z2      (128 cols)   s2 = z1 + Lz2
     hop D: Ls2 = L s2      (128 cols)   out = hw + Ls2 + b2
"""

import sys

for _p in ("/opt/trn_rl_repo", "/root/.axon_site", "/root/.axon_site/_ro/trn_rl_repo",
           "/root/.axon_site/_ro/pypackages"):
    if _p not in sys.path:
        sys.path.append(_p)

import numpy as np
import ml_dtypes

import concourse.bacc as bacc
import concourse.tile as tile
from concourse import bass, mybir
from concourse.bass_utils import run_bass_kernel_spmd
from concourse.masks import make_identity
from concourse import bass_utils as _bu

# walrus disables the LDWEIGHTS fast-load optimization by default; the prop
# sweep here is LDWEIGHTS-bound (one 128-col fp8 weight tile per matmul), so
# flip it on for this kernel's compile.
if not getattr(_bu, "_ldw_patch", False):
    _orig_run_command = _bu.run_command

    def _run_command_ldw(argv, **kw):
        argv = [a
                for a in argv]
        return _orig_run_command(argv, **kw)

    _bu.run_command = _run_command_ldw
    _bu._ldw_patch = True

# problem constants (hardcoded per harness contract)
N, E, IN, HID, OUT, K = 10000, 320000, 256, 256, 128, 3
CORES = 8
NP = 10240          # padded node count
RPC = NP // CORES   # rows per core = 1280
MB = RPC // 128     # M-blocks per core = 10
MBH = MB // 2       # half of the M-blocks = 5
KT = NP // 128      # K-tiles = 80
F = IN              # layer-1 prop width = 256
P = 128

FP8 = mybir.dt.float8e4
BF16 = mybir.dt.bfloat16
F32 = mybir.dt.float32

_STATE = {}


def _build():
    nc = bacc.Bacc("TRN2", target_bir_lowering=False, debug=False, num_devices=CORES)

    # DRAM I/O (per-core data supplied via in_maps)
    at_d = nc.dram_tensor("at", [P, KT * RPC], FP8, kind="ExternalInput")
    xoT_d = nc.dram_tensor("xoT", [F, RPC], BF16, kind="ExternalInput")
    xT_d = nc.dram_tensor("xT", [2, P, NP], BF16, kind="ExternalInput")
    disf_d = nc.dram_tensor("disf", [P, KT], F32, kind="ExternalInput")
    diso_d = nc.dram_tensor("diso", [P, MB], F32, kind="ExternalInput")
    ndiso_d = nc.dram_tensor("ndiso", [P, MB], F32, kind="ExternalInput")
    # w1x = [W1[0]-W1[2], W1[1], 2*W1[2]], w2x likewise for W2
    w1x_d = nc.dram_tensor("w1x", [K, IN, HID], BF16, kind="ExternalInput")
    w2x_d = nc.dram_tensor("w2x", [K, HID, OUT], BF16, kind="ExternalInput")
    b1r_d = nc.dram_tensor("b1r", [P, HID], F32, kind="ExternalInput")
    b2r_d = nc.dram_tensor("b2r", [P, OUT], F32, kind="ExternalInput")
    out_d = nc.dram_tensor("outo", [RPC, OUT], F32, kind="ExternalOutput")

    xoT_r = xoT_d.ap().rearrange("(c p) d -> c p d", p=P)

    with tile.TileContext(nc) as tc:
        with (
            tc.tile_pool(name="res", bufs=1) as res,
            tc.tile_pool(name="wrk", bufs=1) as wrk,
            tc.tile_pool(name="pprop", bufs=1, space="PSUM") as pprop,
            tc.tile_pool(name="pterm", bufs=1, space="PSUM") as pterm,
            tc.tile_pool(name="ptr", bufs=1, space="PSUM") as ptr,
            tc.tile_pool(name="dram", bufs=1, space="DRAM") as dram,
        ):
            # ---- small loads first: xoT + weights feed the pre-hop matmuls ----
            xoT_t = []
            for c in range(2):
                t = res.tile([P, RPC], BF16, tag=f"xoT{c}", name=f"xoT{c}")
                nc.sync.dma_start(t[:], xoT_r[c])
                xoT_t.append(t)
            w1t = [[None, None] for _ in range(K)]
            for k in range(K):
                for c in range(2):
                    t = res.tile([P, HID], BF16, tag=f"w1_{k}_{c}", name=f"w1_{k}_{c}")
                    nc.sync.dma_start(t[:], w1x_d[k, c * P:(c + 1) * P, :])
                    w1t[k][c] = t
            w2t = [[None, None] for _ in range(K)]
            for k in range(K):
                for c in range(2):
                    t = res.tile([P, OUT], BF16, tag=f"w2_{k}_{c}", name=f"w2_{k}_{c}")
                    nc.sync.dma_start(t[:], w2x_d[k, c * P:(c + 1) * P, :])
                    w2t[k][c] = t
            diso = res.tile([P, MB], F32, name="diso")
            nc.sync.dma_start(diso[:], diso_d[:])
            ndiso = res.tile([P, MB], F32, name="ndiso")
            nc.sync.dma_start(ndiso[:], ndiso_d[:])
            b1r = res.tile([P, HID], F32, name="b1r")
            nc.sync.dma_start(b1r[:], b1r_d[:])
            b2r = res.tile([P, OUT], F32, name="b2r")
            nc.sync.dma_start(b2r[:], b2r_d[:])

            disf = res.tile([P, KT], F32, name="disf")
            nc.sync.dma_start(disf[:], disf_d[:])

            # at: one SBUF-resident tile, partition-contiguous DRAM layout ->
            # 8 chunk DMAs with 12.8KB contiguous runs per partition.
            at_res = res.tile([P, KT * RPC], FP8, name="at_res")
            CH = 8
            chw = KT * RPC // CH
            # prop rhs tiles; first filled by the on-device full d2 compute,
            # then refilled by each AllGather round
            u_tiles = [res.tile([P, F], BF16, tag=f"u{kt}", name=f"u{kt}")
                       for kt in range(KT)]

            ident = res.tile([P, P], F32, name="ident")
            make_identity(nc, ident[:])
            idb = res.tile([P, P], BF16, name="idb")
            nc.vector.tensor_copy(idb[:], ident[:])

            # persistent per-block tensors
            d1_t = [res.tile([P, F], F32, tag=f"d1{m}", name=f"d1{m}") for m in range(MB)]
            e0_t = [res.tile([P, F], BF16, tag=f"e0{m}", name=f"e0{m}") for m in range(MB)]
            h_t = [res.tile([P, F], F32, tag=f"h{m}", name=f"h{m}") for m in range(MB)]
            z1_t = [res.tile([P, OUT], F32, tag=f"z1{m}", name=f"z1{m}") for m in range(MB)]
            hw_t = [res.tile([P, OUT], F32, tag=f"hw{m}", name=f"hw{m}") for m in range(MB)]

            # AG bounce buffers: [round][half]; rounds A,B carry 256 cols, C,D 128
            HR = MBH * P
            AGW = [F, OUT, OUT]
            ag_in = [[dram.tile([HR, AGW[i]], BF16, name=f"agin{i}{h}")
                      for h in range(2)] for i in range(3)]
            ag_out = [[dram.tile([CORES * HR, AGW[i]], BF16, name=f"agout{i}{h}")
                       for h in range(2)] for i in range(3)]

            def stage_ag(i, mb, src_ap):
                half, m = mb // MBH, mb % MBH
                nc.sync.dma_start(ag_in[i][half][m * P:(m + 1) * P, :], src_ap)

            def emit_ag(i, half):
                nc.gpsimd.collective_compute(
                    "AllGather", mybir.AluOpType.bypass,
                    replica_groups=[list(range(CORES))],
                    ins=[ag_in[i][half][:].opt()],
                    outs=[ag_out[i][half][:].opt()],
                )

            def reload_u(i):
                n_cols = AGW[i]
                for kt in range(KT):
                    c8, m = kt // MB, kt % MB
                    half, mh = (0, m) if m < MBH else (1, m - MBH)
                    src = ag_out[i][half][c8 * HR + mh * P: c8 * HR + (mh + 1) * P, :]
                    nc.sync.dma_start(u_tiles[kt][:, :n_cols], src)

            def kt_order(split):
                if not split:
                    return list(range(KT))
                return [kt for kt in range(KT) if kt % MB < MBH] + \
                       [kt for kt in range(KT) if kt % MB >= MBH]

            def emit_prop(mb, n_cols, split):
                pp = pprop.tile([P, n_cols], F32, tag="pp", bufs=4, name=f"pp_{mb}")
                sl = slice(mb * P, (mb + 1) * P)
                for j, kt in enumerate(kt_order(split)):
                    nc.tensor.matmul(
                        pp[:], at_tiles[kt][:, sl], u_tiles[kt][:, :n_cols],
                        start=(j == 0), stop=(j == KT - 1),
                    )
                return pp

            def mm6(psum_ap, lhsTs, rhs_pair):
                nc.tensor.matmul(psum_ap, lhsTs[0][:], rhs_pair[0][:], start=True, stop=False)
                nc.tensor.matmul(psum_ap, lhsTs[1][:], rhs_pair[1][:], start=False, stop=True)

            # ---- at chunk loads first: no compute depends on their order,
            # so the DMA engines stream them at full rate in the background.
            for ch in range(CH):
                nc.sync.dma_start(at_res[:, ch * chw:(ch + 1) * chw],
                                  at_d[:, ch * chw:(ch + 1) * chw])
            at_tiles = [at_res[:, kt * RPC:(kt + 1) * RPC] for kt in range(KT)]

            # ---- replicated full d2 = x @ (2 W12): every core computes all
            # NP rows (160 small matmuls) directly into the u tiles, so hop A
            # needs no AllGather at all. Double-buffered 640-col xT chunks.
            HCH = 16
            hw_cols = NP // HCH  # 640
            for hc in range(HCH):
                xTc = [wrk.tile([P, hw_cols], BF16, tag=f"xTc{c}", bufs=2,
                                name=f"xTc{hc}_{c}") for c in range(2)]
                for c in range(2):
                    nc.sync.dma_start(xTc[c][:],
                                      xT_d[c, :, hc * hw_cols:(hc + 1) * hw_cols])
                for m in range(hw_cols // P):
                    kt = (hc * hw_cols) // P + m
                    dp = pterm.tile([P, F], F32, tag="tp", bufs=2, name=f"d2f_{kt}")
                    mm6(dp[:], [xTc[c][:, m * P:(m + 1) * P] for c in range(2)], w1t[2])
                    nc.vector.tensor_scalar_mul(u_tiles[kt][:], dp[:], disf[:, kt:kt + 1])

            # ---- d1, e0 from own-rows x^T ----
            xoT_sl = [[xoT_t[c][:, m * P:(m + 1) * P] for c in range(2)] for m in range(MB)]
            for mb in range(MB):
                dp = pterm.tile([P, F], F32, tag="tp", bufs=2, name=f"d1p_{mb}")
                mm6(dp[:], xoT_sl[mb], w1t[1])
                nc.vector.tensor_copy(d1_t[mb][:], dp[:])
                e0p = ptr.tile([P, F], F32, tag="e0p", bufs=2, name=f"e0p_{mb}")
                mm6(e0p[:], xoT_sl[mb], w1t[0])
                nc.vector.tensor_copy(e0_t[mb][:], e0p[:])

            # PE warmup bridge while the tail of the at matrix lands
            wps = ptr.tile([P, P], F32, tag="e0p", bufs=2, name="warm_ps")
            for w in range(160):
                nc.tensor.matmul(wps[:], idb[:], idb[:], start=(w == 0),
                                 stop=(w == 159))

            # ---- hop A: Ld2 = L d2 ; s1 = d1 + Ld2 -> stage ----
            for mb in range(MB):
                pp = emit_prop(mb, F, split=True)
                s1 = wrk.tile([P, F], F32, tag="s1", bufs=2, name=f"s1_{mb}")
                nc.vector.tensor_scalar_mul(s1[:], pp[:], ndiso[:, mb:mb + 1])
                nc.vector.tensor_add(s1[:], s1[:], d1_t[mb][:])
                sc = wrk.tile([P, F], BF16, tag="sc", bufs=3, name=f"scB_{mb}")
                nc.vector.tensor_scalar_mul(sc[:], s1[:], diso[:, mb:mb + 1])
                stage_ag(0, mb, sc[:])
                if mb == MBH - 1:
                    emit_ag(0, 0)
            emit_ag(0, 1)
            reload_u(0)

            # ---- hop B: Ls1 ; h = relu(e0 + Ls1 + b1); z1, z2(staged), hw ----
            for mb in range(MB):
                pp = emit_prop(mb, F, split=True)
                h = h_t[mb]
                nc.vector.tensor_scalar_mul(h[:], pp[:], ndiso[:, mb:mb + 1])
                nc.vector.tensor_add(h[:], h[:], e0_t[mb][:])
                nc.vector.tensor_add(h[:], h[:], b1r[:])
                nc.vector.tensor_scalar_max(h[:], h[:], 0.0)
                # h^T via PE transpose (bf16) for the layer-2 feature matmuls
                hT = []
                for c in range(2):
                    tps = ptr.tile([P, P], F32, tag="e0p", bufs=2, name=f"hTp_{mb}_{c}")
                    nc.tensor.transpose(tps[:], h[:, c * P:(c + 1) * P], ident[:])
                    tb = wrk.tile([P, P], BF16, tag="hTsb", bufs=4, name=f"hTs_{mb}_{c}")
                    nc.vector.tensor_copy(tb[:], tps[:])
                    hT.append(tb)
                zp = pterm.tile([P, OUT], F32, tag="tp", bufs=2, name=f"z1p_{mb}")
                mm6(zp[:], hT, w2t[1])
                nc.vector.tensor_copy(z1_t[mb][:], zp[:])
                z2p = pterm.tile([P, OUT], F32, tag="tp", bufs=2, name=f"z2p_{mb}")
                mm6(z2p[:], hT, w2t[2])
                sc = wrk.tile([P, OUT], BF16, tag="scC", bufs=3, name=f"scC_{mb}")
                nc.vector.tensor_scalar_mul(sc[:], z2p[:], diso[:, mb:mb + 1])
                stage_ag(1, mb, sc[:])
                hwp = ptr.tile([P, OUT], F32, tag="e0p", bufs=2, name=f"hwp_{mb}")
                mm6(hwp[:], hT, w2t[0])
                nc.vector.tensor_copy(hw_t[mb][:], hwp[:])
                if mb == MBH - 1:
                    emit_ag(1, 0)
            emit_ag(1, 1)
            reload_u(1)

            # ---- hops C and D run transposed: the 128-col feature tile is
            # the stationary operand (1 LDWEIGHTS per k-tile instead of 10)
            # and the adjacency streams as the moving operand in <=512-wide
            # PSUM chunks. Output [feat, dst] is PE-transposed back per block.
            CHK = [(0, 512), (512, 512), (1024, 256)]

            def emit_prop_T(tagn):
                ppc = [pprop.tile([P, 512], F32, tag="pp", bufs=4,
                                  name=f"{tagn}_{i}") for i in range(3)]
                for j, kt in enumerate(kt_order(True)):
                    for i, (off, w) in enumerate(CHK):
                        nc.tensor.matmul(
                            ppc[i][:, :w], u_tiles[kt][:, :OUT],
                            at_tiles[kt][:, off:off + w],
                            start=(j == 0), stop=(j == KT - 1),
                        )
                # evict [feat, dst] to SBUF for re-transposition
                sT = wrk.tile([P, RPC], F32, tag="sT", bufs=1, name=f"{tagn}_s")
                for i, (off, w) in enumerate(CHK):
                    nc.vector.tensor_copy(sT[:, off:off + w], ppc[i][:, :w])
                return sT

            def block_T(sT, mb, tagn):
                # transpose [feat, dst-block] back to node-major psum block
                tps = ptr.tile([P, P], F32, tag="e0p", bufs=2, name=f"{tagn}p_{mb}")
                nc.tensor.transpose(tps[:], sT[:, mb * P:(mb + 1) * P], ident[:])
                return tps

            # ---- hop C: Lz2 ; s2 = z1 + Lz2 -> stage ----
            sT = emit_prop_T("ppc")
            for mb in range(MB):
                tps = block_T(sT, mb, "trC")
                s2 = wrk.tile([P, OUT], F32, tag="s2", bufs=2, name=f"s2_{mb}")
                nc.vector.tensor_scalar_mul(s2[:], tps[:], ndiso[:, mb:mb + 1])
                nc.vector.tensor_add(s2[:], s2[:], z1_t[mb][:])
                sc = wrk.tile([P, OUT], BF16, tag="scC", bufs=3, name=f"scD_{mb}")
                nc.vector.tensor_scalar_mul(sc[:], s2[:], diso[:, mb:mb + 1])
                stage_ag(2, mb, sc[:])
                if mb == MBH - 1:
                    emit_ag(2, 0)
            emit_ag(2, 1)
            reload_u(2)

            # ---- hop D: Ls2 ; out = hw + Ls2 + b2 ----
            sT2 = emit_prop_T("ppd")
            for mb in range(MB):
                tps = block_T(sT2, mb, "trD")
                oacc = wrk.tile([P, OUT], F32, tag="oacc", bufs=3, name=f"oacc_{mb}")
                nc.vector.tensor_scalar_mul(oacc[:], tps[:], ndiso[:, mb:mb + 1])
                nc.vector.tensor_add(oacc[:], oacc[:], hw_t[mb][:])
                nc.vector.tensor_add(oacc[:], oacc[:], b2r[:])
                nc.sync.dma_start(out_d[mb * P:(mb + 1) * P, :], oacc[:])

    nc.compile()
    return nc


def _prepare_inputs(x, edge, W1, b1, W2, b2):
    x = np.asarray(x, np.float32)
    edge = np.asarray(edge)
    W1 = np.asarray(W1, np.float32)
    b1 = np.asarray(b1, np.float32)
    W2 = np.asarray(W2, np.float32)
    b2 = np.asarray(b2, np.float32)
    src = edge[0].astype(np.int64)
    dst = edge[1].astype(np.int64)

    deg = np.bincount(dst, minlength=N).astype(np.float32)
    dis = np.where(deg > 0, 1.0 / np.sqrt(np.maximum(deg, 1.0)), 0.0).astype(np.float32)

    # dense transposed adjacency counts AT[s, d]
    flat = src * NP + dst
    uniq, cnt = np.unique(flat, return_counts=True)
    at8 = np.zeros(NP * NP, dtype=ml_dtypes.float8_e4m3)
    at8[uniq] = cnt.astype(ml_dtypes.float8_e4m3)
    at8 = at8.reshape(NP, NP)

    dis_pad = np.zeros(NP, np.float32)
    dis_pad[:N] = dis
    x_pad = np.zeros((NP, F), np.float32)
    x_pad[:N] = x

    w1x = np.stack([W1[0] - W1[2], W1[1], 2.0 * W1[2]]).astype(ml_dtypes.bfloat16)
    w2x = np.stack([W2[0] - W2[2], W2[1], 2.0 * W2[2]]).astype(ml_dtypes.bfloat16)
    b1r = np.broadcast_to(b1, (P, HID)).copy()
    b2r = np.broadcast_to(b2, (P, OUT)).copy()

    xTb = np.ascontiguousarray(x_pad.T).astype(ml_dtypes.bfloat16).reshape(2, P, NP)
    disf_h = np.ascontiguousarray(dis_pad.reshape(KT, P).T)
    in_maps = []
    for c in range(CORES):
        rows = slice(c * RPC, (c + 1) * RPC)
        dv = dis_pad[rows]
        atc = np.ascontiguousarray(
            at8[:, rows].reshape(KT, P, RPC).transpose(1, 0, 2).reshape(P, KT * RPC))
        m = {
            "at": atc,
            "xoT": np.ascontiguousarray(x_pad[rows].T).astype(ml_dtypes.bfloat16),
            "xT": xTb,
            "disf": disf_h,
            "diso": np.ascontiguousarray(dv.reshape(MB, P).T),
            "ndiso": np.ascontiguousarray((-dv).reshape(MB, P).T),
            "w1x": w1x,
            "w2x": w2x,
            "b1r": b1r,
            "b2r": b2r,
        }
        in_maps.append(m)
    return in_maps


def _run(in_maps, trace=False, **kw):
    if "nc" not in _STATE:
        _STATE["nc"] = _build()
    r = run_bass_kernel_spmd(_STATE["nc"], in_maps, core_ids=list(range(CORES)),
                             trace=trace, **kw)
    out = np.concatenate([r.results[c]["outo"] for c in range(CORES)], axis=0)
    return out[:N], r


def kernel(**inputs) -> np.ndarray:
    in_maps = _prepare_inputs(**inputs)
    out, _ = _run(in_maps)
    return out


# revision 22
# speedup vs baseline: 1.0324x; 1.0324x over previous
"""ChebyNet (K=3, 2 layers) forward on 8 Trainium2 NeuronCores.

Strategy: node sharding. Each core owns 1280 padded rows (10000 -> 10240).
The sparse propagation  L = -D^-1/2 A D^-1/2  is computed as a dense matmul
against the transposed adjacency-count matrix AT[s, d], held SBUF-resident in
fp8e4m3 (counts are small ints -> exact, partition-contiguous DRAM layout for
fast load). Features move in bf16, accumulation in fp32 PSUM, diagonal
scalings as per-partition scalar multiplies on the vector engine. Between
hops the scaled features are AllGathered across the 8 cores; each AllGather
is split into two half-shard collectives overlapped with compute.

Both layers are restructured using linearity of L (it commutes with the
feature-dimension matmuls), so each hop propagates the minimum column count
and layer 1 needs no on-device transposes:

  Layer 1:  h = relu( x(W10-W12) + L( x W11 + L(x 2W12) ) + b1 )
     d1 = x@W11, d2 = x@(2 W12), e0 = x@(W10-W12)   (from host-side x^T)
     hop A: Ld2 = L d2      (256 cols)   s1 = d1 + Ld2
     hop B: Ls1 = L s1      (256 cols)   h = relu(e0 + Ls1 + b1)
  Layer 2:  out = h(W20-W22) + L( h W21 + L(h 2W22) ) + b2
     z1 = h@W21, z2 = h@(2 W22), hw = h@(W20-W22)   (from PE-transposed h)
     hop C: Lz2 = L z2      (128 cols)   s2 = z1 + Lz2
     hop D: Ls2 = L s2      (128 cols)   out = hw + Ls2 + b2
"""

import sys

for _p in ("/opt/trn_rl_repo", "/root/.axon_site", "/root/.axon_site/_ro/trn_rl_repo",
           "/root/.axon_site/_ro/pypackages"):
    if _p not in sys.path:
        sys.path.append(_p)

import numpy as np
import ml_dtypes

import concourse.bacc as bacc
import concourse.tile as tile
from concourse import bass, mybir
from concourse.bass_utils import run_bass_kernel_spmd
from concourse.masks import make_identity
from concourse import bass_utils as _bu

# walrus disables the LDWEIGHTS fast-load optimization by default; the prop
# sweep here is LDWEIGHTS-bound (one 128-col fp8 weight tile per matmul), so
# flip it on for this kernel's compile.
if not getattr(_bu, "_ldw_patch", False):
    _orig_run_command = _bu.run_command

    def _run_command_ldw(argv, **kw):
        argv = [a
                for a in argv]
        return _orig_run_command(argv, **kw)

    _bu.run_command = _run_command_ldw
    _bu._ldw_patch = True

# problem constants (hardcoded per harness contract)
N, E, IN, HID, OUT, K = 10000, 320000, 256, 256, 128, 3
CORES = 8
NP = 10240          # padded node count
RPC = NP // CORES   # rows per core = 1280
MB = RPC // 128     # M-blocks per core = 10
MBH = MB // 2       # half of the M-blocks = 5
KT = NP // 128      # K-tiles = 80
F = IN              # layer-1 prop width = 256
P = 128

FP8 = mybir.dt.float8e4
BF16 = mybir.dt.bfloat16
F32 = mybir.dt.float32

_STATE = {}


def _build():
    nc = bacc.Bacc("TRN2", target_bir_lowering=False, debug=False, num_devices=CORES)

    # DRAM I/O (per-core data supplied via in_maps)
    at_d = nc.dram_tensor("at", [P, KT * RPC], FP8, kind="ExternalInput")
    xoT_d = nc.dram_tensor("xoT", [F, RPC], BF16, kind="ExternalInput")
    xT_d = nc.dram_tensor("xT", [2, P, NP], BF16, kind="ExternalInput")
    disf_d = nc.dram_tensor("disf", [P, KT], F32, kind="ExternalInput")
    diso_d = nc.dram_tensor("diso", [P, MB], F32, kind="ExternalInput")
    ndiso_d = nc.dram_tensor("ndiso", [P, MB], F32, kind="ExternalInput")
    # w1x = [W1[0]-W1[2], W1[1], 2*W1[2]], w2x likewise for W2
    w1x_d = nc.dram_tensor("w1x", [K, IN, HID], BF16, kind="ExternalInput")
    w2x_d = nc.dram_tensor("w2x", [K, HID, OUT], BF16, kind="ExternalInput")
    b1r_d = nc.dram_tensor("b1r", [P, HID], F32, kind="ExternalInput")
    b2r_d = nc.dram_tensor("b2r", [P, OUT], F32, kind="ExternalInput")
    out_d = nc.dram_tensor("outo", [RPC, OUT], F32, kind="ExternalOutput")

    xoT_r = xoT_d.ap().rearrange("(c p) d -> c p d", p=P)

    with tile.TileContext(nc) as tc:
        with (
            tc.tile_pool(name="res", bufs=1) as res,
            tc.tile_pool(name="wrk", bufs=1) as wrk,
            tc.tile_pool(name="pprop", bufs=1, space="PSUM") as pprop,
            tc.tile_pool(name="pterm", bufs=1, space="PSUM") as pterm,
            tc.tile_pool(name="ptr", bufs=1, space="PSUM") as ptr,
            tc.tile_pool(name="dram", bufs=1, space="DRAM") as dram,
        ):
            # ---- small loads first: xoT + weights feed the pre-hop matmuls ----
            xoT_t = []
            for c in range(2):
                t = res.tile([P, RPC], BF16, tag=f"xoT{c}", name=f"xoT{c}")
                nc.sync.dma_start(t[:], xoT_r[c])
                xoT_t.append(t)
            w1t = [[None, None] for _ in range(K)]
            for k in range(K):
                for c in range(2):
                    t = res.tile([P, HID], BF16, tag=f"w1_{k}_{c}", name=f"w1_{k}_{c}")
                    nc.sync.dma_start(t[:], w1x_d[k, c * P:(c + 1) * P, :])
                    w1t[k][c] = t
            w2t = [[None, None] for _ in range(K)]
            for k in range(K):
                for c in range(2):
                    t = res.tile([P, OUT], BF16, tag=f"w2_{k}_{c}", name=f"w2_{k}_{c}")
                    nc.sync.dma_start(t[:], w2x_d[k, c * P:(c + 1) * P, :])
                    w2t[k][c] = t
            diso = res.tile([P, MB], F32, name="diso")
            nc.sync.dma_start(diso[:], diso_d[:])
            ndiso = res.tile([P, MB], F32, name="ndiso")
            nc.sync.dma_start(ndiso[:], ndiso_d[:])
            b1r = res.tile([P, HID], F32, name="b1r")
            nc.sync.dma_start(b1r[:], b1r_d[:])
            b2r = res.tile([P, OUT], F32, name="b2r")
            nc.sync.dma_start(b2r[:], b2r_d[:])

            disf = res.tile([P, KT], F32, name="disf")
            nc.sync.dma_start(disf[:], disf_d[:])

            # at: one SBUF-resident tile, partition-contiguous DRAM layout ->
            # 8 chunk DMAs with 12.8KB contiguous runs per partition.
            at_res = res.tile([P, KT * RPC], FP8, name="at_res")
            CH = 8
            chw = KT * RPC // CH
            # prop rhs tiles; first filled by the on-device full d2 compute,
            # then refilled by each AllGather round
            u_tiles = [res.tile([P, F], BF16, tag=f"u{kt}", name=f"u{kt}")
                       for kt in range(KT)]

            ident = res.tile([P, P], F32, name="ident")
            make_identity(nc, ident[:])
            idb = res.tile([P, P], BF16, name="idb")
            nc.vector.tensor_copy(idb[:], ident[:])

            # persistent per-block tensors
            d1_t = [res.tile([P, F], F32, tag=f"d1{m}", name=f"d1{m}") for m in range(MB)]
            e0_t = [res.tile([P, F], BF16, tag=f"e0{m}", name=f"e0{m}") for m in range(MB)]
            h_t = [res.tile([P, F], F32, tag=f"h{m}", name=f"h{m}") for m in range(MB)]
            z1_t = [res.tile([P, OUT], F32, tag=f"z1{m}", name=f"z1{m}") for m in range(MB)]
            hw_t = [res.tile([P, OUT], F32, tag=f"hw{m}", name=f"hw{m}") for m in range(MB)]

            # AG bounce buffers: [round][half]; rounds A,B carry 256 cols, C,D 128
            HR = MBH * P
            AGW = [F, OUT, OUT]
            ag_in = [[dram.tile([HR, AGW[i]], BF16, name=f"agin{i}{h}")
                      for h in range(2)] for i in range(3)]
            ag_out = [[dram.tile([CORES * HR, AGW[i]], BF16, name=f"agout{i}{h}")
                       for h in range(2)] for i in range(3)]

            # tiny dummy collective issued first: absorbs the one-time
            # collective-engine bootstrap cost while the CC stream is idle
            dumi = dram.tile([P, 16], BF16, name="dumi")
            dumo = dram.tile([CORES * P, 16], BF16, name="dumo")
            nc.sync.dma_start(dumi[:], xT_d[0, :, 0:16])
            nc.gpsimd.collective_compute(
                "AllGather", mybir.AluOpType.bypass,
                replica_groups=[list(range(CORES))],
                ins=[dumi[:].opt()], outs=[dumo[:].opt()],
            )

            def stage_ag(i, mb, src_ap):
                half, m = mb // MBH, mb % MBH
                nc.sync.dma_start(ag_in[i][half][m * P:(m + 1) * P, :], src_ap)

            def emit_ag(i, half):
                nc.gpsimd.collective_compute(
                    "AllGather", mybir.AluOpType.bypass,
                    replica_groups=[list(range(CORES))],
                    ins=[ag_in[i][half][:].opt()],
                    outs=[ag_out[i][half][:].opt()],
                )

            def reload_u(i):
                n_cols = AGW[i]
                for kt in range(KT):
                    c8, m = kt // MB, kt % MB
                    half, mh = (0, m) if m < MBH else (1, m - MBH)
                    src = ag_out[i][half][c8 * HR + mh * P: c8 * HR + (mh + 1) * P, :]
                    nc.sync.dma_start(u_tiles[kt][:, :n_cols], src)

            def kt_order(split):
                if not split:
                    return list(range(KT))
                return [kt for kt in range(KT) if kt % MB < MBH] + \
                       [kt for kt in range(KT) if kt % MB >= MBH]

            def emit_prop(mb, n_cols, split):
                pp = pprop.tile([P, n_cols], F32, tag="pp", bufs=4, name=f"pp_{mb}")
                sl = slice(mb * P, (mb + 1) * P)
                for j, kt in enumerate(kt_order(split)):
                    nc.tensor.matmul(
                        pp[:], at_tiles[kt][:, sl], u_tiles[kt][:, :n_cols],
                        start=(j == 0), stop=(j == KT - 1),
                    )
                return pp

            def mm6(psum_ap, lhsTs, rhs_pair):
                nc.tensor.matmul(psum_ap, lhsTs[0][:], rhs_pair[0][:], start=True, stop=False)
                nc.tensor.matmul(psum_ap, lhsTs[1][:], rhs_pair[1][:], start=False, stop=True)

            # ---- at chunk loads first: no compute depends on their order,
            # so the DMA engines stream them at full rate in the background.
            for ch in range(CH):
                nc.sync.dma_start(at_res[:, ch * chw:(ch + 1) * chw],
                                  at_d[:, ch * chw:(ch + 1) * chw])
            at_tiles = [at_res[:, kt * RPC:(kt + 1) * RPC] for kt in range(KT)]

            # ---- replicated full d2 = x @ (2 W12): every core computes all
            # NP rows (160 small matmuls) directly into the u tiles, so hop A
            # needs no AllGather at all. Double-buffered 640-col xT chunks.
            HCH = 16
            hw_cols = NP // HCH  # 640
            for hc in range(HCH):
                xTc = [wrk.tile([P, hw_cols], BF16, tag=f"xTc{c}", bufs=2,
                                name=f"xTc{hc}_{c}") for c in range(2)]
                for c in range(2):
                    nc.sync.dma_start(xTc[c][:],
                                      xT_d[c, :, hc * hw_cols:(hc + 1) * hw_cols])
                for m in range(hw_cols // P):
                    kt = (hc * hw_cols) // P + m
                    dp = pterm.tile([P, F], F32, tag="tp", bufs=2, name=f"d2f_{kt}")
                    mm6(dp[:], [xTc[c][:, m * P:(m + 1) * P] for c in range(2)], w1t[2])
                    nc.vector.tensor_scalar_mul(u_tiles[kt][:], dp[:], disf[:, kt:kt + 1])

            # ---- d1, e0 from own-rows x^T ----
            xoT_sl = [[xoT_t[c][:, m * P:(m + 1) * P] for c in range(2)] for m in range(MB)]
            for mb in range(MB):
                dp = pterm.tile([P, F], F32, tag="tp", bufs=2, name=f"d1p_{mb}")
                mm6(dp[:], xoT_sl[mb], w1t[1])
                nc.vector.tensor_copy(d1_t[mb][:], dp[:])
                e0p = ptr.tile([P, F], F32, tag="e0p", bufs=2, name=f"e0p_{mb}")
                mm6(e0p[:], xoT_sl[mb], w1t[0])
                nc.vector.tensor_copy(e0_t[mb][:], e0p[:])

            # PE warmup bridge while the tail of the at matrix lands
            wps = ptr.tile([P, P], F32, tag="e0p", bufs=2, name="warm_ps")
            for w in range(400):
                nc.tensor.matmul(wps[:], idb[:], idb[:], start=(w == 0),
                                 stop=(w == 399))

            # ---- hop A: Ld2 = L d2 ; s1 = d1 + Ld2 -> stage ----
            for mb in range(MB):
                pp = emit_prop(mb, F, split=True)
                s1 = wrk.tile([P, F], F32, tag="s1", bufs=2, name=f"s1_{mb}")
                nc.vector.tensor_scalar_mul(s1[:], pp[:], ndiso[:, mb:mb + 1])
                nc.vector.tensor_add(s1[:], s1[:], d1_t[mb][:])
                sc = wrk.tile([P, F], BF16, tag="sc", bufs=3, name=f"scB_{mb}")
                nc.vector.tensor_scalar_mul(sc[:], s1[:], diso[:, mb:mb + 1])
                stage_ag(0, mb, sc[:])
                if mb == MBH - 1:
                    emit_ag(0, 0)
            emit_ag(0, 1)
            reload_u(0)

            # ---- hop B: Ls1 ; h = relu(e0 + Ls1 + b1); z1, z2(staged), hw ----
            for mb in range(MB):
                pp = emit_prop(mb, F, split=True)
                h = h_t[mb]
                nc.vector.tensor_scalar_mul(h[:], pp[:], ndiso[:, mb:mb + 1])
                nc.vector.tensor_add(h[:], h[:], e0_t[mb][:])
                nc.vector.tensor_add(h[:], h[:], b1r[:])
                nc.vector.tensor_scalar_max(h[:], h[:], 0.0)
                # h^T via PE transpose (bf16) for the layer-2 feature matmuls
                hT = []
                for c in range(2):
                    tps = ptr.tile([P, P], F32, tag="e0p", bufs=2, name=f"hTp_{mb}_{c}")
                    nc.tensor.transpose(tps[:], h[:, c * P:(c + 1) * P], ident[:])
                    tb = wrk.tile([P, P], BF16, tag="hTsb", bufs=4, name=f"hTs_{mb}_{c}")
                    nc.vector.tensor_copy(tb[:], tps[:])
                    hT.append(tb)
                zp = pterm.tile([P, OUT], F32, tag="tp", bufs=2, name=f"z1p_{mb}")
                mm6(zp[:], hT, w2t[1])
                nc.vector.tensor_copy(z1_t[mb][:], zp[:])
                z2p = pterm.tile([P, OUT], F32, tag="tp", bufs=2, name=f"z2p_{mb}")
                mm6(z2p[:], hT, w2t[2])
                sc = wrk.tile([P, OUT], BF16, tag="scC", bufs=3, name=f"scC_{mb}")
                nc.vector.tensor_scalar_mul(sc[:], z2p[:], diso[:, mb:mb + 1])
                stage_ag(1, mb, sc[:])
                hwp = ptr.tile([P, OUT], F32, tag="e0p", bufs=2, name=f"hwp_{mb}")
                mm6(hwp[:], hT, w2t[0])
                nc.vector.tensor_copy(hw_t[mb][:], hwp[:])
                if mb == MBH - 1:
                    emit_ag(1, 0)
            emit_ag(1, 1)
            reload_u(1)

            # ---- hops C and D run transposed: the 128-col feature tile is
            # the stationary operand (1 LDWEIGHTS per k-tile instead of 10)
            # and the adjacency streams as the moving operand in <=512-wide
            # PSUM chunks. Output [feat, dst] is PE-transposed back per block.
            CHK = [(0, 512), (512, 512), (1024, 256)]

            def emit_prop_T(tagn):
                ppc = [pprop.tile([P, 512], F32, tag="pp", bufs=4,
                                  name=f"{tagn}_{i}") for i in range(3)]
                for j, kt in enumerate(kt_order(True)):
                    for i, (off, w) in enumerate(CHK):
                        nc.tensor.matmul(
                            ppc[i][:, :w], u_tiles[kt][:, :OUT],
                            at_tiles[kt][:, off:off + w],
                            start=(j == 0), stop=(j == KT - 1),
                        )
                # evict [feat, dst] to SBUF for re-transposition
                sT = wrk.tile([P, RPC], F32, tag="sT", bufs=1, name=f"{tagn}_s")
                for i, (off, w) in enumerate(CHK):
                    nc.vector.tensor_copy(sT[:, off:off + w], ppc[i][:, :w])
                return sT

            def block_T(sT, mb, tagn):
                # transpose [feat, dst-block] back to node-major psum block
                tps = ptr.tile([P, P], F32, tag="e0p", bufs=2, name=f"{tagn}p_{mb}")
                nc.tensor.transpose(tps[:], sT[:, mb * P:(mb + 1) * P], ident[:])
                return tps

            # ---- hop C: Lz2 ; s2 = z1 + Lz2 -> stage ----
            sT = emit_prop_T("ppc")
            for mb in range(MB):
                tps = block_T(sT, mb, "trC")
                s2 = wrk.tile([P, OUT], F32, tag="s2", bufs=2, name=f"s2_{mb}")
                nc.vector.tensor_scalar_mul(s2[:], tps[:], ndiso[:, mb:mb + 1])
                nc.vector.tensor_add(s2[:], s2[:], z1_t[mb][:])
                sc = wrk.tile([P, OUT], BF16, tag="scC", bufs=3, name=f"scD_{mb}")
                nc.vector.tensor_scalar_mul(sc[:], s2[:], diso[:, mb:mb + 1])
                stage_ag(2, mb, sc[:])
                if mb == MBH - 1:
                    emit_ag(2, 0)
            emit_ag(2, 1)
            reload_u(2)

            # ---- hop D: Ls2 ; out = hw + Ls2 + b2 ----
            sT2 = emit_prop_T("ppd")
            for mb in range(MB):
                tps = block_T(sT2, mb, "trD")
                oacc = wrk.tile([P, OUT], F32, tag="oacc", bufs=3, name=f"oacc_{mb}")
                nc.vector.tensor_scalar_mul(oacc[:], tps[:], ndiso[:, mb:mb + 1])
                nc.vector.tensor_add(oacc[:], oacc[:], hw_t[mb][:])
                nc.vector.tensor_add(oacc[:], oacc[:], b2r[:])
                nc.sync.dma_start(out_d[mb * P:(mb + 1) * P, :], oacc[:])

    nc.compile()
    return nc


def _prepare_inputs(x, edge, W1, b1, W2, b2):
    x = np.asarray(x, np.float32)
    edge = np.asarray(edge)
    W1 = np.asarray(W1, np.float32)
    b1 = np.asarray(b1, np.float32)
    W2 = np.asarray(W2, np.float32)
    b2 = np.asarray(b2, np.float32)
    src = edge[0].astype(np.int64)
    dst = edge[1].astype(np.int64)

    deg = np.bincount(dst, minlength=N).astype(np.float32)
    dis = np.where(deg > 0, 1.0 / np.sqrt(np.maximum(deg, 1.0)), 0.0).astype(np.float32)

    # dense transposed adjacency counts AT[s, d]
    flat = src * NP + dst
    uniq, cnt = np.unique(flat, return_counts=True)
    at8 = np.zeros(NP * NP, dtype=ml_dtypes.float8_e4m3)
    at8[uniq] = cnt.astype(ml_dtypes.float8_e4m3)
    at8 = at8.reshape(NP, NP)

    dis_pad = np.zeros(NP, np.float32)
    dis_pad[:N] = dis
    x_pad = np.zeros((NP, F), np.float32)
    x_pad[:N] = x

    w1x = np.stack([W1[0] - W1[2], W1[1], 2.0 * W1[2]]).astype(ml_dtypes.bfloat16)
    w2x = np.stack([W2[0] - W2[2], W2[1], 2.0 * W2[2]]).astype(ml_dtypes.bfloat16)
    b1r = np.broadcast_to(b1, (P, HID)).copy()
    b2r = np.broadcast_to(b2, (P, OUT)).copy()

    xTb = np.ascontiguousarray(x_pad.T).astype(ml_dtypes.bfloat16).reshape(2, P, NP)
    disf_h = np.ascontiguousarray(dis_pad.reshape(KT, P).T)
    in_maps = []
    for c in range(CORES):
        rows = slice(c * RPC, (c + 1) * RPC)
        dv = dis_pad[rows]
        atc = np.ascontiguousarray(
            at8[:, rows].reshape(KT, P, RPC).transpose(1, 0, 2).reshape(P, KT * RPC))
        m = {
            "at": atc,
            "xoT": np.ascontiguousarray(x_pad[rows].T).astype(ml_dtypes.bfloat16),
            "xT": xTb,
            "disf": disf_h,
            "diso": np.ascontiguousarray(dv.reshape(MB, P).T),
            "ndiso": np.ascontiguousarray((-dv).reshape(MB, P).T),
            "w1x": w1x,
            "w2x": w2x,
            "b1r": b1r,
            "b2r": b2r,
        }
        in_maps.append(m)
    return in_maps


def _run(in_maps, trace=False, **kw):
    if "nc" not in _STATE:
        _STATE["nc"] = _build()
    r = run_bass_kernel_spmd(_STATE["nc"], in_maps, core_ids=list(range(CORES)),
                             trace=trace, **kw)
    out = np.concatenate([r.results[c]["outo"] for c in range(CORES)], axis=0)
    return out[:N], r


def kernel(**inputs) -> np.ndarray:
    in_maps = _prepare_inputs(**inputs)
    out, _ = _run(in_maps)
    return out
